# revision 71
# baseline (speedup 1.0000x reference)
"""Trainium2 Bass kernel for nn_MultiHeadAttention_69106023793143.

Reference computation (B=4, S=2048, D=1024, H=16, HD=64):
    qh = split_heads(q @ Wq + bq); kh, vh likewise
    out = merge_heads(sigmoid((qh @ kh^T) / sqrt(HD)) @ vh)

Sharding (8 cores): core c handles batch b = c//2 and the half = c%2 slice of
the feature axis (512 features = 8 heads).  Projections are tensor-parallel on
the output dim of Wq/Wk/Wv; attention is head-parallel.  The final [B,S,D]
output is assembled host-side from the per-core [2048, 512] natural blocks.

Device strategy per core (vs the f32r baseline, ~380us -> ~358us):
  - Projections in float32r: K and the first Q chunk run as a serial
    prefix; V and Q chunks 1-3 are injectable tasks drained into the early
    attention rounds so the sigmoid engine never idles long.  Projection
    PSUM accumulators ride in the score-tile slots.
  - Q^T/K^T are quantized on evacuation to fp8e4m3 in a pair-packed layout
    (partition p = head-sub*32 + pr, slots i=0/1 in the free dim hold
    d=2*pr+i); the projection W columns are permuted host-side so each
    projection series lands directly in that layout.  Score matmuls then
    run in fp8 DoubleRow mode (2 contraction rows per partition, 0.5
    cycles/row: half the PE cost of f32r), 4 heads sharing the PE rows via
    32-row tile_position groups.
  - sigmoid on ScalarE from PSUM, one 2-bank wave (one k-tile x two heads,
    N=1024) per ACTIVATE, 1/sqrt(HD) folded into ACT's scale.
  - AV matmuls consume attn^T as the STATIONARY operand: out[q, d] +=
    a_t.T @ V-tile with free dim = 64 (bf16 V keeps 1.0 cycles/row below
    the 256-row f32r threshold), so the AV stream charges 64 rows/matmul
    instead of 512: PE attention cost drops 2x.  All 8 series of a (qc,
    hp) round accumulate into ONE PSUM bank ([128, 4, 128]); exactly one
    start=True (zeroes the 2KB zero-region) and one stop=True.
  - AV emission is readiness-ordered and flows across round boundaries
    (PSUM accumulation is order-independent), so the in-order PE stream
    never stalls on a late sigmoid; output evacuation is deferred into the
    next round and runs on DVE, with the DMA in natural [tok, feat] layout.
  - End-to-end max rel err ~5e-3 (fp8 score operands + bf16 attn/V).
  - Nonzero biases are folded in by augmenting the contraction dim with a
    ones-row (host-side, KT=9); with zero biases (the spec'd case) no
    padding is used.
"""

import sys

if "/opt/trn_rl_repo" not in sys.path:
    sys.path.insert(0, "/opt/trn_rl_repo")

from contextlib import ExitStack

import numpy as np

import concourse.tile as tile
from concourse import bacc, mybir
from concourse.bass_utils import run_bass_kernel_spmd

B, S, D, H = 4, 2048, 1024, 16
HD = D // H  # 64
OF = D // 2  # 512 features (8 heads) per core
N_CORES = 8
P = 128
TOK_T = S // P  # 16 token tiles
QC = S // 512  # 4 query chunks of 512
HP = 4  # head pairs per core
F32 = mybir.dt.float32
F32R = mybir.dt.float32r
BF16 = mybir.dt.bfloat16
FP8 = mybir.dt.float8e4  # e4m3

# number of (kt, head) S-tile jobs per (head-pair, q-chunk) per ACTIVATE.
# 2 jobs = one 2-bank PSUM wave (1024-elem ACT instructions); 3-bank waves
# amortize ACT overhead better in the cost model but mis-executed on the
# fake_nrt path, so stay at 2.
WAVE = 2
ACT_AV_LAG = 3   # AV trail (waves) for ACT-routed sigmoids
POOL_AV_LAG = 7  # AV trail for the longer DVE->Pool chain

# When True, the projection inputs (x^T and W) are shipped and multiplied in
# bfloat16: halves the serial prefix DMA (~27 MiB -> ~13.5 MiB) at the cost of
# ~10x higher (but still small) output error. Default off: fp32/float32r
# everywhere gives ~2.5e-4 max rel err.
BF16_INPUTS = False

_cache: dict = {}

# results of the most recent run (exec time etc.), for test harnesses
last_results = None

# ---- custom fused-DVE sigmoid approximation ----
# DVE waves compute the unclamped saturated-curvature cubic
#     t = (min(x*x, CAP)*B + A)*x + 0.5
# on the raw scores x (1/sqrt(HD) folded into A/B/CAP) in ONE DVE
# instruction; the idle GPSIMD/Pool engine then applies clamp01 (it may not
# touch PSUM, but t lives in SBUF).  sigma(x/8) is approximated to ~0.0033
# weighted-rms / 0.04 max err; ACT waves use the true sigmoid.
SIG_A = 0.2411235 / 8
SIG_B = -0.0119587 / 512
SIG_CAP = 64 * 7.4870063
# k-tiles whose sigmoid waves use the DVE/Pool cubic approximation (fixed
# set so the +0.5 correction's V-mass is precomputable once).  Empty: the
# measured engine balance favors the exact ACT sigmoid for every wave (the
# standard-instruction approx chain costs more DVE time than it saves on
# ACT), and the error margin vs the 2e-2 gate stays ~4x.
APPROX_KT = ()

_SIG_OP = None


def _sigmoid_dve_op():
    global _SIG_OP
    if _SIG_OP is not None:
        return _SIG_OP
    import concourse.dve_ops as dvo
    from concourse.dve_spec import (C0, C1, C2, Spec, Src0, Src1, lower,
                                    minn, sq)
    from concourse.dve_uop import DveOpSpec

    name = "SIGTAIL_CUBIC_ANT"
    for op in dvo.OPS:
        if op.name == name:
            _SIG_OP = op
            return op
    body = (minn(sq(Src0), C2) * C1 + C0) * Src0 + Src1

    def ref(in0, in1, c0, c1, c2):
        x = in0.astype(np.float32)
        return (np.minimum(x * x, c2) * c1 + c0) * x + in1

    spec = Spec(body=body, reference=ref)
    opcode = max(dvo._SUB_OPCODE_FOR_NAME.values()) + 1
    shas = {}
    for ver in ("v3", "v4"):
        try:
            uops = lower(spec, ver=ver)
        except ValueError:
            continue
        shas[ver] = DveOpSpec(name=name, opcode=opcode, uops=uops,
                              rd1_en=True).sha(ver)
    op = dvo.DveOp(name, spec, subdim=False, uops_sha=shas)
    dvo.OPS.append(op)
    dvo.CUSTOM_DVE_SPECS[name] = spec
    dvo._SUB_OPCODE_FOR_NAME[name] = opcode
    _SIG_OP = op
    return op


def _build(KT: int):
    """Build the SPMD Bass program. KT = contraction k-tiles (8, or 9 when
    biases are folded in via an augmented ones-row)."""
    nc = bacc.Bacc("TRN2", target_bir_lowering=False, debug=False,
                   num_devices=N_CORES, name="mha_sig")

    KA = KT * P  # augmented contraction size
    XDT = mybir.dt.bfloat16 if BF16_INPUTS else F32R
    xq = nc.dram_tensor("xq", [KA, S], XDT, kind="ExternalInput")
    xk = nc.dram_tensor("xk", [KA, S], XDT, kind="ExternalInput")
    xv = nc.dram_tensor("xv", [KA, S], XDT, kind="ExternalInput")
    wq = nc.dram_tensor("wq", [KA, OF], XDT, kind="ExternalInput")
    wk = nc.dram_tensor("wk", [KA, OF], XDT, kind="ExternalInput")
    wv = nc.dram_tensor("wv", [KA, OF], XDT, kind="ExternalInput")
    # natural [token, feature] layout: the AV matmuls produce out[q, d]
    o_nat = nc.dram_tensor("o_nat", [S, OF], F32, kind="ExternalOutput")
    # view for per-(qc, hp, qs) output stores: [p, qc, qs, hp, j]
    o_nat_r = o_nat.rearrange("(qc qs p) (hp j) -> p qc qs hp j",
                              qs=4, p=P, hp=HP)

    # the augmented (KT=9) layout is bigger; drop x-chunk buffering to fit
    xbufs = 3 if KT == 8 else 2

    xq_r = xq.rearrange("(kt p) t -> p kt t", p=P)
    xk_r = xk.rearrange("(kt p) t -> p kt t", p=P)
    xv_r = xv.rearrange("(kt p) t -> p kt t", p=P)

    with tile.TileContext(nc) as tc:
        with ExitStack() as ctx:
            persist = ctx.enter_context(tc.tile_pool(name="persist", bufs=1))
            wpool = ctx.enter_context(tc.tile_pool(name="wpool", bufs=1))
            xpool = ctx.enter_context(tc.tile_pool(name="xpool", bufs=2))
            ps_pool = ctx.enter_context(
                tc.tile_pool(name="ps_pool", bufs=2, space="PSUM"))
            apool = ctx.enter_context(tc.tile_pool(name="apool", bufs=6))
            opool = ctx.enter_context(tc.tile_pool(name="opool", bufs=2))

            # constants for the +0.5 correction of centered approx waves:
            # halfcol sums 0.5*V over the approx k-tiles, ones_row broadcasts
            # the result over all 128 q-partitions of a round
            halfcol = persist.tile([P, 1], BF16)
            nc.vector.memset(halfcol[:], 0.5)
            ones_row = persist.tile([1, P], BF16)
            nc.vector.memset(ones_row[:], 1.0)
            vpart_sb = persist.tile([1, OF], BF16)

            def emit_sigmoid(a_t, st, nwave, kt, tag):
                """ACT waves compute the true sigmoid; waves with kt in
                APPROX_KT get the saturated-curvature cubic via standard
                DVE/Pool instructions, producing CENTERED attn-0.5 in
                [-1/2, 1/2] (the +0.5 mass is restored by a rank-1
                correction matmul per round).  Returns True if ACT."""
                sig_ctr_started[0] = True
                if kt not in APPROX_KT:
                    nc.scalar.activation(
                        out=a_t[:, :nwave, :],
                        in_=st[:, :nwave, :],
                        func=mybir.ActivationFunctionType.Sigmoid,
                        scale=1.0 / np.sqrt(HD).item(),
                    )
                    return True
                y_cp = apool.tile([P, WAVE, 512], BF16, tag="y_cp", bufs=2,
                                  name=f"ycp_{tag}")
                nc.vector.tensor_copy(out=y_cp[:, :nwave, :],
                                      in_=st[:, :nwave, :])
                u_sq = apool.tile([P, WAVE, 512], BF16, tag="u_sq", bufs=2,
                                  name=f"usq_{tag}")
                nc.vector.tensor_tensor(
                    out=u_sq[:, :nwave, :], in0=y_cp[:, :nwave, :],
                    in1=y_cp[:, :nwave, :], op=mybir.AluOpType.mult)
                w_t = apool.tile([P, WAVE, 512], BF16, tag="w_t", bufs=2,
                                 name=f"wt_{tag}")
                nc.vector.tensor_scalar(
                    out=w_t[:, :nwave, :], in0=u_sq[:, :nwave, :],
                    scalar1=SIG_CAP, scalar2=SIG_B,
                    op0=mybir.AluOpType.min, op1=mybir.AluOpType.mult)
                v_t = apool.tile([P, WAVE, 512], BF16, tag="v_t", bufs=2,
                                 name=f"vt_{tag}")
                nc.vector.scalar_tensor_tensor(
                    out=v_t[:, :nwave, :], in0=w_t[:, :nwave, :],
                    scalar=SIG_A, in1=st[:, :nwave, :],
                    op0=mybir.AluOpType.add, op1=mybir.AluOpType.mult)
                nc.gpsimd.tensor_scalar(
                    out=a_t[:, :nwave, :], in0=v_t[:, :nwave, :],
                    scalar1=-0.5, scalar2=0.5,
                    op0=mybir.AluOpType.max, op1=mybir.AluOpType.min)
                return False

            # --- persistent weights + projection outputs ---
            # (each W is DMA'd right before the projection phase that uses it
            # so the serial prefix DMA stream isn't front-loaded with all
            # three weight tensors)
            wk_sb = persist.tile([P, KT, OF], XDT)
            wk_r = wk.rearrange("(kt p) n -> p kt n", p=P)
            for m in range(HP):
                # 4 column-chunk DMAs: the first K projection series only
                # needs columns 0:128, so the PE starts ~5us earlier
                nc.sync.dma_start(wk_sb[:, :, m * P:(m + 1) * P],
                                  wk_r[:, :, m * P:(m + 1) * P])
            wv_sb = persist.tile([P, KT, OF], XDT)
            wq_sb = persist.tile([P, KT, OF], XDT)

            # K^T / Q^T in fp8, pair-packed for DoubleRow score matmuls:
            # partition p = hsub*32 + pr (hsub = head-in-group), slot i in
            # the free dim; element [p, g, i, t] = X^T[head g*4+hsub,
            # d = 2*pr + i, t].  The projection W columns are permuted
            # host-side so each (g, i) series lands directly in this layout.
            # V: [tok-in-tile, kt, of] bf16.
            k_dr = persist.tile([P, 2, 2, S], FP8)
            q_dr = persist.tile([P, 2, 2, S], FP8)
            v_sb = persist.tile([P, TOK_T, OF], BF16)

            # projection PSUM accumulators ride in the scores-tag slots (the
            # attention score tiles are idle during the projection phase and
            # PSUM has no room for dedicated accumulators once scores run
            # triple-buffered)
            pacc_ctr = [0]

            def proj_acc(name):
                pacc_ctr[0] += 1
                t = ps_pool.tile([P, WAVE, 512], F32, tag="scores", bufs=3,
                                 name=name)
                return t

            copy_ctr = [0]
            sig_ctr_started = [False]  # set once attention waves begin

            def evac_copy(out, in_):
                # PSUM evacuation: DVE-heavy (ACT is the sigmoid bottleneck);
                # every third copy goes to ACT only during the projection
                # prefix, where ACT would otherwise idle
                copy_ctr[0] += 1
                if sig_ctr_started[0] or copy_ctr[0] % 3:
                    nc.vector.tensor_copy(out=out, in_=in_)
                else:
                    nc.scalar.copy(out=out, in_=in_)

            def proj_transposed(x_r, w_sb, dst_of, tc_idx, label, width=512):
                """dst[:, m, tc*width:+width] = (W-slice).T @ x-chunk
                ([of, tok])."""
                x_tile = xpool.tile([P, KT, width], XDT, tag="xchunk",
                                    bufs=xbufs, name=f"x_{label}_{tc_idx}")
                nc.sync.dma_start(
                    x_tile[:],
                    x_r[:, :, tc_idx * width:(tc_idx + 1) * width])
                for m in range(HP):
                    ps = proj_acc(f"ps_{label}_{tc_idx}_{m}")
                    for kt in range(KT):
                        nc.tensor.matmul(
                            ps[:, 0, :width],
                            lhsT=w_sb[:, kt, m * P:(m + 1) * P],
                            rhs=x_tile[:, kt, :],
                            start=(kt == 0),
                            stop=(kt == KT - 1),
                        )
                    evac_copy(dst_of(m), ps[:, 0, :width])

            def proj_v(hc_idx):
                """v_sb[:, hc*2+m, :] = x-half-chunk.T @ Wv  ([tok, of])."""
                x_tile = xpool.tile([P, KT, 256], XDT, tag="xchunk", bufs=xbufs,
                                    name=f"x_v_{hc_idx}")
                nc.sync.dma_start(x_tile[:],
                                  xv_r[:, :, hc_idx * 256:(hc_idx + 1) * 256])
                for m in range(2):
                    ps = proj_acc(f"ps_v_{hc_idx}_{m}")
                    for kt in range(KT):
                        nc.tensor.matmul(
                            ps[:, 0, :],
                            lhsT=x_tile[:, kt, m * P:(m + 1) * P],
                            rhs=wv_sb[:, kt, :],
                            start=(kt == 0),
                            stop=(kt == KT - 1),
                        )
                    evac_copy(v_sb[:, hc_idx * 2 + m, :], ps[:, 0, :])

            # K projections and the first Q chunk gate attention round 0 and
            # run as a serial prefix; V and Q chunks 1-3 become injectable
            # tasks drained into the early attention rounds so the sigmoid
            # engines never sit idle behind the projection phase.
            for tc_idx in range(QC):
                proj_transposed(
                    xk_r, wk_sb,
                    lambda m, _tc=tc_idx: k_dr[:, m // 2, m % 2,
                                               _tc * 512:(_tc + 1) * 512],
                    tc_idx, "k")
            for m in range(HP):
                nc.sync.dma_start(
                    wq_sb[:, :, m * P:(m + 1) * P],
                    wq.rearrange("(kt p) n -> p kt n", p=P)[:, :,
                                                           m * P:(m + 1) * P])
            proj_transposed(
                xq_r, wq_sb,
                lambda m: q_dr[:, m // 2, m % 2, 0:512], 0, "q")
            nc.sync.dma_start(wv_sb[:], wv.rearrange("(kt p) n -> p kt n", p=P))

            # --- deferred projection task queue ---
            proj_tasks = []

            def v_dma_task(hc_idx):
                def run():
                    x_tile = xpool.tile([P, KT, 256], XDT, tag="xchunk",
                                        bufs=xbufs, name=f"x_v_{hc_idx}")
                    nc.sync.dma_start(
                        x_tile[:],
                        xv_r[:, :, hc_idx * 256:(hc_idx + 1) * 256])
                    v_tiles[hc_idx] = x_tile
                return run

            def v_series_task(hc_idx, m):
                def run():
                    x_tile = v_tiles[hc_idx]
                    ps = proj_acc(f"ps_v_{hc_idx}_{m}")
                    for kt in range(KT):
                        nc.tensor.matmul(
                            ps[:, 0, :],
                            lhsT=x_tile[:, kt, m * P:(m + 1) * P],
                            rhs=wv_sb[:, kt, :],
                            start=(kt == 0),
                            stop=(kt == KT - 1),
                        )
                    evac_copy(v_sb[:, hc_idx * 2 + m, :], ps[:, 0, :])
                return run

            def q_dma_task(tc_idx):
                def run():
                    x_tile = xpool.tile([P, KT, 512], XDT, tag="xchunk",
                                        bufs=xbufs, name=f"x_q_{tc_idx}")
                    nc.sync.dma_start(
                        x_tile[:],
                        xq_r[:, :, tc_idx * 512:(tc_idx + 1) * 512])
                    q_tiles[tc_idx] = x_tile
                return run

            def q_series_task(tc_idx, m):
                def run():
                    x_tile = q_tiles[tc_idx]
                    ps = proj_acc(f"ps_q_{tc_idx}_{m}")
                    for kt in range(KT):
                        nc.tensor.matmul(
                            ps[:, 0, :],
                            lhsT=wq_sb[:, kt, m * P:(m + 1) * P],
                            rhs=x_tile[:, kt, :],
                            start=(kt == 0),
                            stop=(kt == KT - 1),
                        )
                    evac_copy(
                        q_dr[:, m // 2, m % 2,
                             tc_idx * 512:(tc_idx + 1) * 512],
                        ps[:, 0, :])
                return run

            def vpart_task():
                # vpart[of] = 0.5 * sum over approx k-tiles of V rows; the
                # per-round correction matmuls broadcast it across q
                if not APPROX_KT:
                    return
                ps = proj_acc("ps_vpart")
                for i, kt in enumerate(APPROX_KT):
                    nc.tensor.matmul(
                        ps[0:1, 0, :],
                        lhsT=halfcol[:, :],
                        rhs=v_sb[:, kt, :],
                        start=(i == 0),
                        stop=(i == len(APPROX_KT) - 1),
                    )
                nc.vector.tensor_copy(out=vpart_sb[:, :], in_=ps[0:1, 0, :])

            v_tiles: dict = {}
            q_tiles: dict = {}
            for hc in range(8):
                proj_tasks.append(v_dma_task(hc))
                proj_tasks.append(v_series_task(hc, 0))
                proj_tasks.append(v_series_task(hc, 1))
            proj_tasks.append(vpart_task)
            for tc_idx in range(1, QC):
                proj_tasks.append(q_dma_task(tc_idx))
                for m in range(HP):
                    proj_tasks.append(q_series_task(tc_idx, m))

            def drain_proj(n):
                for _ in range(n):
                    if proj_tasks:
                        proj_tasks.pop(0)()

            # jobs per (hp, qc): (kt, head) pairs
            jobs = [(kt, h) for kt in range(TOK_T) for h in range(2)]
            waves = [jobs[i:i + WAVE] for i in range(0, len(jobs), WAVE)]

            # deferred output evacuation: (o_acc, qc, hp) whose copy + DMA
            # are emitted a few waves into the NEXT round, so the engine
            # queues never head-of-line-block the next round's sigmoids
            pending_out = []

            def flush_pending_out():
                while pending_out:
                    po_acc, po_qc, po_hp = pending_out.pop(0)
                    o_sb = opool.tile([P, 4, P], F32, tag="o_sb",
                                      name=f"osb_{po_qc}_{po_hp}")
                    nc.vector.tensor_copy(out=o_sb[:], in_=po_acc[:])
                    nc.sync.dma_start(o_nat_r[:, po_qc, :, po_hp, :], o_sb[:])

            # --- attention rounds with cross-round AV flow ---
            # AV matmuls drain by expected a_t readiness (global wave index),
            # flowing freely across round boundaries so a late approx-chain
            # a_t never stalls the round tail: the next round's scores and
            # sigmoids proceed while the stragglers land.
            pending = []  # (ready_gw, seq, wave, a_t, rctx)
            seq_ctr = [0]

            def emit_avs(wave, a_t, rctx):
                o_acc, r_hp = rctx["o_acc"], rctx["hp"]
                for j, (kt, h) in enumerate(wave):
                    for qs in range(4):
                        # out[q, d] += attn^T-tile.T @ V-tile
                        nc.tensor.matmul(
                            o_acc[:, qs, h * HD:(h + 1) * HD],
                            lhsT=a_t[:, j, qs * P:(qs + 1) * P],
                            rhs=v_sb[:, kt,
                                     r_hp * P + h * HD:
                                     r_hp * P + (h + 1) * HD],
                            start=(rctx["emitted"] == 0),
                            stop=(not APPROX_KT
                                  and rctx["emitted"] == rctx["total"] - 1),
                        )
                        rctx["emitted"] += 1
                if rctx["emitted"] == rctx["total"]:
                    if APPROX_KT:
                        # restore the +0.5 mass the centered approx waves
                        # dropped (rank-1 over q); last one closes the group
                        for qs in range(4):
                            nc.tensor.matmul(
                                o_acc[:, qs, :],
                                lhsT=ones_row[0:1, :],
                                rhs=vpart_sb[0:1, r_hp * P:(r_hp + 1) * P],
                                start=False,
                                stop=(qs == 3),
                            )
                    pending_out.append((o_acc, rctx["qc"], r_hp))

            def service_pending(now_gw):
                pending.sort()
                while pending and pending[0][0] <= now_gw:
                    _, _, w_, at_, rctx_ = pending.pop(0)
                    emit_avs(w_, at_, rctx_)

            gw = 0
            for qc in range(QC):
                for hp in range(HP):
                    o_acc = ps_pool.tile([P, 4, P], F32, tag="oacc", bufs=2,
                                         name=f"oacc_{qc}_{hp}")
                    rctx = {"o_acc": o_acc, "qc": qc, "hp": hp,
                            "emitted": 0, "total": len(waves) * WAVE * 4}
                    for wi, wave in enumerate(waves):
                        st = ps_pool.tile([P, WAVE, 512], F32, tag="scores",
                                       bufs=3, name=f"st_{qc}_{hp}_{wi}")
                        for j, (kt, h) in enumerate(wave):
                            # scores^T tile: [k-tokens, q-tokens] for head
                            # 2hp+h via fp8 DoubleRow: contraction d = 64
                            # packed as 32 partitions x 2 slots
                            g, hsub = hp // 2, 2 * (hp % 2) + h
                            nc.tensor.matmul(
                                st[:, j, :],
                                lhsT=k_dr[hsub * 32:(hsub + 1) * 32, g, :,
                                          kt * P:(kt + 1) * P],
                                rhs=q_dr[hsub * 32:(hsub + 1) * 32, g, :,
                                         qc * 512:(qc + 1) * 512],
                                start=True,
                                stop=True,
                                perf_mode=mybir.MatmulPerfMode.DoubleRow,
                                tile_position=(hsub * 32, 0),
                            )
                        a_t = apool.tile([P, WAVE, 512], BF16, tag="a_t",
                                         bufs=14, name=f"a_{qc}_{hp}_{wi}")
                        is_act = emit_sigmoid(a_t, st, len(wave), wave[0][0],
                                              f"{qc}_{hp}_{wi}")
                        # drain deferred projections: V inside round 0 (its
                        # AVs need kt progressively), Q chunks trickled
                        # through later rounds ahead of their q-chunk
                        if qc == 0 and hp == 0:
                            if wi < 12:
                                drain_proj(2)
                        elif wi in (2, 10):
                            drain_proj(1)
                        if wi == 2:
                            # previous rounds' outputs leave PSUM only now:
                            # their copies never queue ahead of this round's
                            # sigmoid work on DVE
                            flush_pending_out()
                        ready = gw + (ACT_AV_LAG if is_act else POOL_AV_LAG)
                        seq_ctr[0] += 1
                        pending.append((ready, seq_ctr[0], wave, a_t, rctx))
                        service_pending(gw)
                        gw += 1
            service_pending(10 ** 9)
            flush_pending_out()

    nc.compile()
    return nc


def _prep_core_inputs(q, k, v, Wq, bq, Wk, bk, Wv, bv, KT):
    """Host-side shard + transpose. Returns in_maps for 8 cores."""
    KA = KT * P
    aug = KA > D
    if BF16_INPUTS:
        import ml_dtypes
        xdt = ml_dtypes.bfloat16
    else:
        xdt = np.float32

    def x_t(x_b):  # [S, D] -> [KA, S]
        xt = np.ascontiguousarray(x_b.T)  # [D, S]
        if not aug:
            return xt.astype(xdt)
        out = np.zeros((KA, S), xdt)
        out[:D] = xt
        out[D] = 1.0
        return out

    # Q/K weight columns are permuted so projection series m = (g, i) lands
    # directly in the pair-packed fp8 layout the DoubleRow score matmuls
    # read: series m, PSUM partition p <- of-column (g*4 + p//32)*64 +
    # 2*(p%32) + i of the core's half.
    perm = np.empty(OF, np.int64)
    for m in range(4):
        g, i = divmod(m, 2)
        p = np.arange(P)
        perm[m * P + p] = (g * 4 + p // 32) * 64 + 2 * (p % 32) + i

    def w_slice(W, b, half, permute=False):  # -> [KA, OF]
        ws = W[:, half * OF:(half + 1) * OF]
        bs = b[half * OF:(half + 1) * OF]
        if permute:
            ws = ws[:, perm]
            bs = bs[perm]
        if not aug:
            return np.ascontiguousarray(ws).astype(xdt)
        out = np.zeros((KA, OF), xdt)
        out[:D] = ws
        out[D] = bs
        return out

    xts = {}
    in_maps = []
    for c in range(N_CORES):
        b, half = divmod(c, 2)
        if b not in xts:
            xts[b] = (x_t(q[b]), x_t(k[b]), x_t(v[b]))
        xq_c, xk_c, xv_c = xts[b]
        in_maps.append({
            "xq": xq_c,
            "xk": xk_c,
            "xv": xv_c,
            "wq": w_slice(Wq, bq, half, permute=True),
            "wk": w_slice(Wk, bk, half, permute=True),
            "wv": w_slice(Wv, bv, half),
        })
    return in_maps


def kernel(q, k, v, Wq, bq, Wk, bk, Wv, bv):
    global last_results
    q = np.ascontiguousarray(np.asarray(q, np.float32))
    k = np.ascontiguousarray(np.asarray(k, np.float32))
    v = np.ascontiguousarray(np.asarray(v, np.float32))
    Wq = np.asarray(Wq, np.float32)
    Wk = np.asarray(Wk, np.float32)
    Wv = np.asarray(Wv, np.float32)
    bq = np.asarray(bq, np.float32)
    bk = np.asarray(bk, np.float32)
    bv = np.asarray(bv, np.float32)

    aug = any(np.any(b_) for b_ in (bq, bk, bv))
    KT = (D // P) + (1 if aug else 0)

    key = (KT, BF16_INPUTS)
    if key not in _cache:
        _cache[key] = _build(KT)
    nc = _cache[key]

    in_maps = _prep_core_inputs(q, k, v, Wq, bq, Wk, bk, Wv, bv, KT)
    res = run_bass_kernel_spmd(nc, in_maps, core_ids=list(range(N_CORES)))
    last_results = res

    out = np.empty((B, S, D), np.float32)
    for c in range(N_CORES):
        b, half = divmod(c, 2)
        out[b, :, half * OF:(half + 1) * OF] = res.results[c]["o_nat"]
    return out



# revision 73
# speedup vs baseline: 1.0090x; 1.0090x over previous
"""Trainium2 Bass kernel for nn_MultiHeadAttention_69106023793143.

Reference computation (B=4, S=2048, D=1024, H=16, HD=64):
    qh = split_heads(q @ Wq + bq); kh, vh likewise
    out = merge_heads(sigmoid((qh @ kh^T) / sqrt(HD)) @ vh)

Sharding (8 cores): core c handles batch b = c//2 and the half = c%2 slice of
the feature axis (512 features = 8 heads).  Projections are tensor-parallel on
the output dim of Wq/Wk/Wv; attention is head-parallel.  The final [B,S,D]
output is assembled host-side from the per-core [2048, 512] natural blocks.

Device strategy per core (vs the f32r baseline, ~380us -> ~358us):
  - Projections in float32r: K and the first Q chunk run as a serial
    prefix; V and Q chunks 1-3 are injectable tasks drained into the early
    attention rounds so the sigmoid engine never idles long.  Projection
    PSUM accumulators ride in the score-tile slots.
  - Q^T/K^T are quantized on evacuation to fp8e4m3 in a pair-packed layout
    (partition p = head-sub*32 + pr, slots i=0/1 in the free dim hold
    d=2*pr+i); the projection W columns are permuted host-side so each
    projection series lands directly in that layout.  Score matmuls then
    run in fp8 DoubleRow mode (2 contraction rows per partition, 0.5
    cycles/row: half the PE cost of f32r), 4 heads sharing the PE rows via
    32-row tile_position groups.
  - sigmoid on ScalarE from PSUM, one 2-bank wave (one k-tile x two heads,
    N=1024) per ACTIVATE, 1/sqrt(HD) folded into ACT's scale.
  - AV matmuls consume attn^T as the STATIONARY operand: out[q, d] +=
    a_t.T @ V-tile with free dim = 64 (bf16 V keeps 1.0 cycles/row below
    the 256-row f32r threshold), so the AV stream charges 64 rows/matmul
    instead of 512: PE attention cost drops 2x.  All 8 series of a (qc,
    hp) round accumulate into ONE PSUM bank ([128, 4, 128]); exactly one
    start=True (zeroes the 2KB zero-region) and one stop=True.
  - AV emission is readiness-ordered and flows across round boundaries
    (PSUM accumulation is order-independent), so the in-order PE stream
    never stalls on a late sigmoid; output evacuation is deferred into the
    next round and runs on DVE, with the DMA in natural [tok, feat] layout.
  - End-to-end max rel err ~5e-3 (fp8 score operands + bf16 attn/V).
  - Nonzero biases are folded in by augmenting the contraction dim with a
    ones-row (host-side, KT=9); with zero biases (the spec'd case) no
    padding is used.
"""

import sys

if "/opt/trn_rl_repo" not in sys.path:
    sys.path.insert(0, "/opt/trn_rl_repo")

from contextlib import ExitStack

import numpy as np

import concourse.tile as tile
from concourse import bacc, mybir
from concourse.bass_utils import run_bass_kernel_spmd

B, S, D, H = 4, 2048, 1024, 16
HD = D // H  # 64
OF = D // 2  # 512 features (8 heads) per core
N_CORES = 8
P = 128
TOK_T = S // P  # 16 token tiles
QC = S // 512  # 4 query chunks of 512
HP = 4  # head pairs per core
F32 = mybir.dt.float32
F32R = mybir.dt.float32r
BF16 = mybir.dt.bfloat16
FP8 = mybir.dt.float8e4  # e4m3

# number of (kt, head) S-tile jobs per (head-pair, q-chunk) per ACTIVATE.
# 2 jobs = one 2-bank PSUM wave (1024-elem ACT instructions); 3-bank waves
# amortize ACT overhead better in the cost model but mis-executed on the
# fake_nrt path, so stay at 2.
WAVE = 2
ACT_AV_LAG = 3   # AV trail (waves) for ACT-routed sigmoids
POOL_AV_LAG = 7  # AV trail for the longer DVE->Pool chain

# When True, the projection inputs (x^T and W) are shipped and multiplied in
# bfloat16: halves the serial prefix DMA (~27 MiB -> ~13.5 MiB) at the cost of
# ~10x higher (but still small) output error. Default off: fp32/float32r
# everywhere gives ~2.5e-4 max rel err.
BF16_INPUTS = False

_cache: dict = {}

# results of the most recent run (exec time etc.), for test harnesses
last_results = None

# ---- custom fused-DVE sigmoid approximation ----
# DVE waves compute the unclamped saturated-curvature cubic
#     t = (min(x*x, CAP)*B + A)*x + 0.5
# on the raw scores x (1/sqrt(HD) folded into A/B/CAP) in ONE DVE
# instruction; the idle GPSIMD/Pool engine then applies clamp01 (it may not
# touch PSUM, but t lives in SBUF).  sigma(x/8) is approximated to ~0.0033
# weighted-rms / 0.04 max err; ACT waves use the true sigmoid.
SIG_A = 0.2411235 / 8
SIG_B = -0.0119587 / 512
SIG_CAP = 64 * 7.4870063
# k-tiles whose sigmoid waves use the DVE/Pool cubic approximation (fixed
# set so the +0.5 correction's V-mass is precomputable once).  Empty: the
# measured engine balance favors the exact ACT sigmoid for every wave (the
# standard-instruction approx chain costs more DVE time than it saves on
# ACT), and the error margin vs the 2e-2 gate stays ~4x.
APPROX_KT = ()

_SIG_OP = None


def _sigmoid_dve_op():
    global _SIG_OP
    if _SIG_OP is not None:
        return _SIG_OP
    import concourse.dve_ops as dvo
    from concourse.dve_spec import (C0, C1, C2, Spec, Src0, Src1, lower,
                                    minn, sq)
    from concourse.dve_uop import DveOpSpec

    name = "SIGTAIL_CUBIC_ANT"
    for op in dvo.OPS:
        if op.name == name:
            _SIG_OP = op
            return op
    body = (minn(sq(Src0), C2) * C1 + C0) * Src0 + Src1

    def ref(in0, in1, c0, c1, c2):
        x = in0.astype(np.float32)
        return (np.minimum(x * x, c2) * c1 + c0) * x + in1

    spec = Spec(body=body, reference=ref)
    opcode = max(dvo._SUB_OPCODE_FOR_NAME.values()) + 1
    shas = {}
    for ver in ("v3", "v4"):
        try:
            uops = lower(spec, ver=ver)
        except ValueError:
            continue
        shas[ver] = DveOpSpec(name=name, opcode=opcode, uops=uops,
                              rd1_en=True).sha(ver)
    op = dvo.DveOp(name, spec, subdim=False, uops_sha=shas)
    dvo.OPS.append(op)
    dvo.CUSTOM_DVE_SPECS[name] = spec
    dvo._SUB_OPCODE_FOR_NAME[name] = opcode
    _SIG_OP = op
    return op


def _build(KT: int):
    """Build the SPMD Bass program. KT = contraction k-tiles (8, or 9 when
    biases are folded in via an augmented ones-row)."""
    nc = bacc.Bacc("TRN2", target_bir_lowering=False, debug=False,
                   num_devices=N_CORES, name="mha_sig")

    KA = KT * P  # augmented contraction size
    XDT = mybir.dt.bfloat16 if BF16_INPUTS else F32R
    xq = nc.dram_tensor("xq", [KA, S], XDT, kind="ExternalInput")
    xk = nc.dram_tensor("xk", [KA, S], XDT, kind="ExternalInput")
    xv = nc.dram_tensor("xv", [KA, S], XDT, kind="ExternalInput")
    wq = nc.dram_tensor("wq", [KA, OF], XDT, kind="ExternalInput")
    wk = nc.dram_tensor("wk", [KA, OF], XDT, kind="ExternalInput")
    wv = nc.dram_tensor("wv", [KA, OF], XDT, kind="ExternalInput")
    # natural [token, feature] layout: the AV matmuls produce out[q, d]
    o_nat = nc.dram_tensor("o_nat", [S, OF], F32, kind="ExternalOutput")
    # view for per-(qc, hp, qs) output stores: [p, qc, qs, hp, j]
    o_nat_r = o_nat.rearrange("(qc qs p) (hp j) -> p qc qs hp j",
                              qs=4, p=P, hp=HP)

    # the augmented (KT=9) layout is bigger; drop x-chunk buffering to fit
    xbufs = 3 if KT == 8 else 2

    xq_r = xq.rearrange("(kt p) t -> p kt t", p=P)
    xk_r = xk.rearrange("(kt p) t -> p kt t", p=P)
    xv_r = xv.rearrange("(kt p) t -> p kt t", p=P)

    with tile.TileContext(nc) as tc:
        with ExitStack() as ctx:
            persist = ctx.enter_context(tc.tile_pool(name="persist", bufs=1))
            wpool = ctx.enter_context(tc.tile_pool(name="wpool", bufs=1))
            xpool = ctx.enter_context(tc.tile_pool(name="xpool", bufs=2))
            ps_pool = ctx.enter_context(
                tc.tile_pool(name="ps_pool", bufs=2, space="PSUM"))
            apool = ctx.enter_context(tc.tile_pool(name="apool", bufs=6))
            opool = ctx.enter_context(tc.tile_pool(name="opool", bufs=2))

            # constants for the +0.5 correction of centered approx waves:
            # halfcol sums 0.5*V over the approx k-tiles, ones_row broadcasts
            # the result over all 128 q-partitions of a round
            halfcol = persist.tile([P, 1], BF16)
            nc.vector.memset(halfcol[:], 0.5)
            ones_row = persist.tile([1, P], BF16)
            nc.vector.memset(ones_row[:], 1.0)
            vpart_sb = persist.tile([1, OF], BF16)

            def emit_sigmoid(a_t, st, nwave, kt, tag):
                """ACT waves compute the true sigmoid; waves with kt in
                APPROX_KT get the saturated-curvature cubic via standard
                DVE/Pool instructions, producing CENTERED attn-0.5 in
                [-1/2, 1/2] (the +0.5 mass is restored by a rank-1
                correction matmul per round).  Returns True if ACT."""
                sig_ctr_started[0] = True
                if kt not in APPROX_KT:
                    nc.scalar.activation(
                        out=a_t[:, :nwave, :],
                        in_=st[:, :nwave, :],
                        func=mybir.ActivationFunctionType.Sigmoid,
                        scale=1.0 / np.sqrt(HD).item(),
                    )
                    return True
                y_cp = apool.tile([P, WAVE, 512], BF16, tag="y_cp", bufs=2,
                                  name=f"ycp_{tag}")
                nc.vector.tensor_copy(out=y_cp[:, :nwave, :],
                                      in_=st[:, :nwave, :])
                u_sq = apool.tile([P, WAVE, 512], BF16, tag="u_sq", bufs=2,
                                  name=f"usq_{tag}")
                nc.vector.tensor_tensor(
                    out=u_sq[:, :nwave, :], in0=y_cp[:, :nwave, :],
                    in1=y_cp[:, :nwave, :], op=mybir.AluOpType.mult)
                w_t = apool.tile([P, WAVE, 512], BF16, tag="w_t", bufs=2,
                                 name=f"wt_{tag}")
                nc.vector.tensor_scalar(
                    out=w_t[:, :nwave, :], in0=u_sq[:, :nwave, :],
                    scalar1=SIG_CAP, scalar2=SIG_B,
                    op0=mybir.AluOpType.min, op1=mybir.AluOpType.mult)
                v_t = apool.tile([P, WAVE, 512], BF16, tag="v_t", bufs=2,
                                 name=f"vt_{tag}")
                nc.vector.scalar_tensor_tensor(
                    out=v_t[:, :nwave, :], in0=w_t[:, :nwave, :],
                    scalar=SIG_A, in1=st[:, :nwave, :],
                    op0=mybir.AluOpType.add, op1=mybir.AluOpType.mult)
                nc.gpsimd.tensor_scalar(
                    out=a_t[:, :nwave, :], in0=v_t[:, :nwave, :],
                    scalar1=-0.5, scalar2=0.5,
                    op0=mybir.AluOpType.max, op1=mybir.AluOpType.min)
                return False

            # --- persistent weights + projection outputs ---
            # (each W is DMA'd right before the projection phase that uses it
            # so the serial prefix DMA stream isn't front-loaded with all
            # three weight tensors)
            wk_sb = persist.tile([P, KT, OF], XDT)
            wk_r = wk.rearrange("(kt p) n -> p kt n", p=P)
            for m in range(HP):
                # 4 column-chunk DMAs: the first K projection series only
                # needs columns 0:128, so the PE starts ~5us earlier
                nc.sync.dma_start(wk_sb[:, :, m * P:(m + 1) * P],
                                  wk_r[:, :, m * P:(m + 1) * P])
            wv_sb = persist.tile([P, KT, OF], XDT)
            wq_sb = persist.tile([P, KT, OF], XDT)

            # K^T / Q^T in fp8, pair-packed for DoubleRow score matmuls:
            # partition p = hsub*32 + pr (hsub = head-in-group), slot i in
            # the free dim; element [p, g, i, t] = X^T[head g*4+hsub,
            # d = 2*pr + i, t].  The projection W columns are permuted
            # host-side so each (g, i) series lands directly in this layout.
            # V: [tok-in-tile, kt, of] bf16.
            k_dr = persist.tile([P, 2, 2, S], FP8)
            q_dr = persist.tile([P, 2, 2, S], FP8)
            v_sb = persist.tile([P, TOK_T, OF], BF16)

            # projection PSUM accumulators ride in the scores-tag slots (the
            # attention score tiles are idle during the projection phase and
            # PSUM has no room for dedicated accumulators once scores run
            # triple-buffered)
            pacc_ctr = [0]

            def proj_acc(name):
                pacc_ctr[0] += 1
                t = ps_pool.tile([P, WAVE, 512], F32, tag="scores", bufs=3,
                                 name=name)
                return t

            copy_ctr = [0]
            sig_ctr_started = [False]  # set once attention waves begin

            def evac_copy(out, in_):
                # PSUM evacuation: DVE-heavy (ACT is the sigmoid bottleneck);
                # every third copy goes to ACT only during the projection
                # prefix, where ACT would otherwise idle
                copy_ctr[0] += 1
                if sig_ctr_started[0] or copy_ctr[0] % 3:
                    nc.vector.tensor_copy(out=out, in_=in_)
                else:
                    nc.scalar.copy(out=out, in_=in_)

            def proj_transposed(x_r, w_sb, dst_of, tc_idx, label, width=512):
                """dst[:, m, tc*width:+width] = (W-slice).T @ x-chunk
                ([of, tok])."""
                x_tile = xpool.tile([P, KT, width], XDT, tag="xchunk",
                                    bufs=xbufs, name=f"x_{label}_{tc_idx}")
                nc.sync.dma_start(
                    x_tile[:],
                    x_r[:, :, tc_idx * width:(tc_idx + 1) * width])
                for m in range(HP):
                    ps = proj_acc(f"ps_{label}_{tc_idx}_{m}")
                    for kt in range(KT):
                        nc.tensor.matmul(
                            ps[:, 0, :width],
                            lhsT=w_sb[:, kt, m * P:(m + 1) * P],
                            rhs=x_tile[:, kt, :],
                            start=(kt == 0),
                            stop=(kt == KT - 1),
                        )
                    evac_copy(dst_of(m), ps[:, 0, :width])

            def proj_v(hc_idx):
                """v_sb[:, hc*2+m, :] = x-half-chunk.T @ Wv  ([tok, of])."""
                x_tile = xpool.tile([P, KT, 256], XDT, tag="xchunk", bufs=xbufs,
                                    name=f"x_v_{hc_idx}")
                nc.sync.dma_start(x_tile[:],
                                  xv_r[:, :, hc_idx * 256:(hc_idx + 1) * 256])
                for m in range(2):
                    ps = proj_acc(f"ps_v_{hc_idx}_{m}")
                    for kt in range(KT):
                        nc.tensor.matmul(
                            ps[:, 0, :],
                            lhsT=x_tile[:, kt, m * P:(m + 1) * P],
                            rhs=wv_sb[:, kt, :],
                            start=(kt == 0),
                            stop=(kt == KT - 1),
                        )
                    evac_copy(v_sb[:, hc_idx * 2 + m, :], ps[:, 0, :])

            # K projections and the first Q chunk gate attention round 0 and
            # run as a serial prefix; V and Q chunks 1-3 become injectable
            # tasks drained into the early attention rounds so the sigmoid
            # engines never sit idle behind the projection phase.
            for tc_idx in range(QC):
                proj_transposed(
                    xk_r, wk_sb,
                    lambda m, _tc=tc_idx: k_dr[:, m // 2, m % 2,
                                               _tc * 512:(_tc + 1) * 512],
                    tc_idx, "k")
            for m in range(HP):
                nc.sync.dma_start(
                    wq_sb[:, :, m * P:(m + 1) * P],
                    wq.rearrange("(kt p) n -> p kt n", p=P)[:, :,
                                                           m * P:(m + 1) * P])
            proj_transposed(
                xq_r, wq_sb,
                lambda m: q_dr[:, m // 2, m % 2, 0:512], 0, "q")
            nc.sync.dma_start(wv_sb[:], wv.rearrange("(kt p) n -> p kt n", p=P))

            # --- deferred projection task queue ---
            proj_tasks = []

            def v_dma_task(hc_idx):
                def run():
                    x_tile = xpool.tile([P, KT, 256], XDT, tag="xchunk",
                                        bufs=xbufs, name=f"x_v_{hc_idx}")
                    nc.sync.dma_start(
                        x_tile[:],
                        xv_r[:, :, hc_idx * 256:(hc_idx + 1) * 256])
                    v_tiles[hc_idx] = x_tile
                return run

            def v_series_task(hc_idx, m):
                def run():
                    x_tile = v_tiles[hc_idx]
                    ps = proj_acc(f"ps_v_{hc_idx}_{m}")
                    for kt in range(KT):
                        nc.tensor.matmul(
                            ps[:, 0, :],
                            lhsT=x_tile[:, kt, m * P:(m + 1) * P],
                            rhs=wv_sb[:, kt, :],
                            start=(kt == 0),
                            stop=(kt == KT - 1),
                        )
                    evac_copy(v_sb[:, hc_idx * 2 + m, :], ps[:, 0, :])
                return run

            def q_dma_task(tc_idx):
                def run():
                    x_tile = xpool.tile([P, KT, 512], XDT, tag="xchunk",
                                        bufs=xbufs, name=f"x_q_{tc_idx}")
                    nc.sync.dma_start(
                        x_tile[:],
                        xq_r[:, :, tc_idx * 512:(tc_idx + 1) * 512])
                    q_tiles[tc_idx] = x_tile
                return run

            q_accs: dict = {}

            def q_series_task(tc_idx, m, half):
                # half-series granules (~0.85us of PE each) so a drained
                # projection never starves the ACT sigmoid stream through
                # the 3-slot score buffer
                def run():
                    x_tile = q_tiles[tc_idx]
                    if half == 0:
                        ps = proj_acc(f"ps_q_{tc_idx}_{m}")
                        q_accs[(tc_idx, m)] = ps
                    else:
                        ps = q_accs[(tc_idx, m)]
                    lo = 0 if half == 0 else KT // 2
                    hi = KT // 2 if half == 0 else KT
                    for kt in range(lo, hi):
                        nc.tensor.matmul(
                            ps[:, 0, :],
                            lhsT=wq_sb[:, kt, m * P:(m + 1) * P],
                            rhs=x_tile[:, kt, :],
                            start=(kt == 0),
                            stop=(kt == KT - 1),
                        )
                    if half == 1:
                        evac_copy(
                            q_dr[:, m // 2, m % 2,
                                 tc_idx * 512:(tc_idx + 1) * 512],
                            ps[:, 0, :])
                return run

            def vpart_task():
                # vpart[of] = 0.5 * sum over approx k-tiles of V rows; the
                # per-round correction matmuls broadcast it across q
                if not APPROX_KT:
                    return
                ps = proj_acc("ps_vpart")
                for i, kt in enumerate(APPROX_KT):
                    nc.tensor.matmul(
                        ps[0:1, 0, :],
                        lhsT=halfcol[:, :],
                        rhs=v_sb[:, kt, :],
                        start=(i == 0),
                        stop=(i == len(APPROX_KT) - 1),
                    )
                nc.vector.tensor_copy(out=vpart_sb[:, :], in_=ps[0:1, 0, :])

            v_tiles: dict = {}
            q_tiles: dict = {}
            for hc in range(8):
                proj_tasks.append(v_dma_task(hc))
                proj_tasks.append(v_series_task(hc, 0))
                proj_tasks.append(v_series_task(hc, 1))
            proj_tasks.append(vpart_task)
            for tc_idx in range(1, QC):
                proj_tasks.append(q_dma_task(tc_idx))
                for m in range(HP):
                    proj_tasks.append(q_series_task(tc_idx, m, 0))
                    proj_tasks.append(q_series_task(tc_idx, m, 1))

            def drain_proj(n):
                for _ in range(n):
                    if proj_tasks:
                        proj_tasks.pop(0)()

            # jobs per (hp, qc): (kt, head) pairs
            jobs = [(kt, h) for kt in range(TOK_T) for h in range(2)]
            waves = [jobs[i:i + WAVE] for i in range(0, len(jobs), WAVE)]

            # deferred output evacuation: (o_acc, qc, hp) whose copy + DMA
            # are emitted a few waves into the NEXT round, so the engine
            # queues never head-of-line-block the next round's sigmoids
            pending_out = []

            def flush_pending_out():
                while pending_out:
                    po_acc, po_qc, po_hp = pending_out.pop(0)
                    o_sb = opool.tile([P, 4, P], F32, tag="o_sb",
                                      name=f"osb_{po_qc}_{po_hp}")
                    nc.vector.tensor_copy(out=o_sb[:], in_=po_acc[:])
                    nc.sync.dma_start(o_nat_r[:, po_qc, :, po_hp, :], o_sb[:])

            # --- attention rounds with cross-round AV flow ---
            # AV matmuls drain by expected a_t readiness (global wave index),
            # flowing freely across round boundaries so a late approx-chain
            # a_t never stalls the round tail: the next round's scores and
            # sigmoids proceed while the stragglers land.
            pending = []  # (ready_gw, seq, wave, a_t, rctx)
            seq_ctr = [0]

            def emit_avs(wave, a_t, rctx):
                o_acc, r_hp = rctx["o_acc"], rctx["hp"]
                for j, (kt, h) in enumerate(wave):
                    for qs in range(4):
                        # out[q, d] += attn^T-tile.T @ V-tile
                        nc.tensor.matmul(
                            o_acc[:, qs, h * HD:(h + 1) * HD],
                            lhsT=a_t[:, j, qs * P:(qs + 1) * P],
                            rhs=v_sb[:, kt,
                                     r_hp * P + h * HD:
                                     r_hp * P + (h + 1) * HD],
                            start=(rctx["emitted"] == 0),
                            stop=(not APPROX_KT
                                  and rctx["emitted"] == rctx["total"] - 1),
                        )
                        rctx["emitted"] += 1
                if rctx["emitted"] == rctx["total"]:
                    if APPROX_KT:
                        # restore the +0.5 mass the centered approx waves
                        # dropped (rank-1 over q); last one closes the group
                        for qs in range(4):
                            nc.tensor.matmul(
                                o_acc[:, qs, :],
                                lhsT=ones_row[0:1, :],
                                rhs=vpart_sb[0:1, r_hp * P:(r_hp + 1) * P],
                                start=False,
                                stop=(qs == 3),
                            )
                    pending_out.append((o_acc, rctx["qc"], r_hp))

            def service_pending(now_gw):
                pending.sort()
                while pending and pending[0][0] <= now_gw:
                    _, _, w_, at_, rctx_ = pending.pop(0)
                    emit_avs(w_, at_, rctx_)

            gw = 0
            for qc in range(QC):
                for hp in range(HP):
                    o_acc = ps_pool.tile([P, 4, P], F32, tag="oacc", bufs=2,
                                         name=f"oacc_{qc}_{hp}")
                    rctx = {"o_acc": o_acc, "qc": qc, "hp": hp,
                            "emitted": 0, "total": len(waves) * WAVE * 4}
                    for wi, wave in enumerate(waves):
                        st = ps_pool.tile([P, WAVE, 512], F32, tag="scores",
                                       bufs=3, name=f"st_{qc}_{hp}_{wi}")
                        for j, (kt, h) in enumerate(wave):
                            # scores^T tile: [k-tokens, q-tokens] for head
                            # 2hp+h via fp8 DoubleRow: contraction d = 64
                            # packed as 32 partitions x 2 slots
                            g, hsub = hp // 2, 2 * (hp % 2) + h
                            nc.tensor.matmul(
                                st[:, j, :],
                                lhsT=k_dr[hsub * 32:(hsub + 1) * 32, g, :,
                                          kt * P:(kt + 1) * P],
                                rhs=q_dr[hsub * 32:(hsub + 1) * 32, g, :,
                                         qc * 512:(qc + 1) * 512],
                                start=True,
                                stop=True,
                                perf_mode=mybir.MatmulPerfMode.DoubleRow,
                                tile_position=(hsub * 32, 0),
                            )
                        a_t = apool.tile([P, WAVE, 512], BF16, tag="a_t",
                                         bufs=14, name=f"a_{qc}_{hp}_{wi}")
                        is_act = emit_sigmoid(a_t, st, len(wave), wave[0][0],
                                              f"{qc}_{hp}_{wi}")
                        # drain deferred projections: V inside round 0 (its
                        # AVs need kt progressively), Q chunks trickled
                        # through later rounds ahead of their q-chunk
                        if qc == 0 and hp == 0:
                            if wi < 12:
                                drain_proj(2)
                        elif wi in (2, 6, 10, 14):
                            drain_proj(1)
                        if wi == 2:
                            # previous rounds' outputs leave PSUM only now:
                            # their copies never queue ahead of this round's
                            # sigmoid work on DVE
                            flush_pending_out()
                        ready = gw + (ACT_AV_LAG if is_act else POOL_AV_LAG)
                        seq_ctr[0] += 1
                        pending.append((ready, seq_ctr[0], wave, a_t, rctx))
                        service_pending(gw)
                        gw += 1
            service_pending(10 ** 9)
            flush_pending_out()

    nc.compile()
    return nc


def _prep_core_inputs(q, k, v, Wq, bq, Wk, bk, Wv, bv, KT):
    """Host-side shard + transpose. Returns in_maps for 8 cores."""
    KA = KT * P
    aug = KA > D
    if BF16_INPUTS:
        import ml_dtypes
        xdt = ml_dtypes.bfloat16
    else:
        xdt = np.float32

    def x_t(x_b):  # [S, D] -> [KA, S]
        xt = np.ascontiguousarray(x_b.T)  # [D, S]
        if not aug:
            return xt.astype(xdt)
        out = np.zeros((KA, S), xdt)
        out[:D] = xt
        out[D] = 1.0
        return out

    # Q/K weight columns are permuted so projection series m = (g, i) lands
    # directly in the pair-packed fp8 layout the DoubleRow score matmuls
    # read: series m, PSUM partition p <- of-column (g*4 + p//32)*64 +
    # 2*(p%32) + i of the core's half.
    perm = np.empty(OF, np.int64)
    for m in range(4):
        g, i = divmod(m, 2)
        p = np.arange(P)
        perm[m * P + p] = (g * 4 + p // 32) * 64 + 2 * (p % 32) + i

    def w_slice(W, b, half, permute=False):  # -> [KA, OF]
        ws = W[:, half * OF:(half + 1) * OF]
        bs = b[half * OF:(half + 1) * OF]
        if permute:
            ws = ws[:, perm]
            bs = bs[perm]
        if not aug:
            return np.ascontiguousarray(ws).astype(xdt)
        out = np.zeros((KA, OF), xdt)
        out[:D] = ws
        out[D] = bs
        return out

    xts = {}
    in_maps = []
    for c in range(N_CORES):
        b, half = divmod(c, 2)
        if b not in xts:
            xts[b] = (x_t(q[b]), x_t(k[b]), x_t(v[b]))
        xq_c, xk_c, xv_c = xts[b]
        in_maps.append({
            "xq": xq_c,
            "xk": xk_c,
            "xv": xv_c,
            "wq": w_slice(Wq, bq, half, permute=True),
            "wk": w_slice(Wk, bk, half, permute=True),
            "wv": w_slice(Wv, bv, half),
        })
    return in_maps


def kernel(q, k, v, Wq, bq, Wk, bk, Wv, bv):
    global last_results
    q = np.ascontiguousarray(np.asarray(q, np.float32))
    k = np.ascontiguousarray(np.asarray(k, np.float32))
    v = np.ascontiguousarray(np.asarray(v, np.float32))
    Wq = np.asarray(Wq, np.float32)
    Wk = np.asarray(Wk, np.float32)
    Wv = np.asarray(Wv, np.float32)
    bq = np.asarray(bq, np.float32)
    bk = np.asarray(bk, np.float32)
    bv = np.asarray(bv, np.float32)

    aug = any(np.any(b_) for b_ in (bq, bk, bv))
    KT = (D // P) + (1 if aug else 0)

    key = (KT, BF16_INPUTS)
    if key not in _cache:
        _cache[key] = _build(KT)
    nc = _cache[key]

    in_maps = _prep_core_inputs(q, k, v, Wq, bq, Wk, bk, Wv, bv, KT)
    res = run_bass_kernel_spmd(nc, in_maps, core_ids=list(range(N_CORES)))
    last_results = res

    out = np.empty((B, S, D), np.float32)
    for c in range(N_CORES):
        b, half = divmod(c, 2)
        out[b, :, half * OF:(half + 1) * OF] = res.results[c]["o_nat"]
    return out



# revision 75
# speedup vs baseline: 1.0317x; 1.0225x over previous
"""Trainium2 Bass kernel for nn_MultiHeadAttention_69106023793143.

Reference computation (B=4, S=2048, D=1024, H=16, HD=64):
    qh = split_heads(q @ Wq + bq); kh, vh likewise
    out = merge_heads(sigmoid((qh @ kh^T) / sqrt(HD)) @ vh)

Sharding (8 cores): core c handles batch b = c//2 and the half = c%2 slice of
the feature axis (512 features = 8 heads).  Projections are tensor-parallel on
the output dim of Wq/Wk/Wv; attention is head-parallel.  The final [B,S,D]
output is assembled host-side from the per-core [2048, 512] natural blocks.

Device strategy per core (vs the f32r baseline, ~380us -> ~358us):
  - Projections in float32r: K and the first Q chunk run as a serial
    prefix; V and Q chunks 1-3 are injectable tasks drained into the early
    attention rounds so the sigmoid engine never idles long.  Projection
    PSUM accumulators ride in the score-tile slots.
  - Q^T/K^T are quantized on evacuation to fp8e4m3 in a pair-packed layout
    (partition p = head-sub*32 + pr, slots i=0/1 in the free dim hold
    d=2*pr+i); the projection W columns are permuted host-side so each
    projection series lands directly in that layout.  Score matmuls then
    run in fp8 DoubleRow mode (2 contraction rows per partition, 0.5
    cycles/row: half the PE cost of f32r), 4 heads sharing the PE rows via
    32-row tile_position groups.
  - sigmoid on ScalarE from PSUM, one 2-bank wave (one k-tile x two heads,
    N=1024) per ACTIVATE, 1/sqrt(HD) folded into ACT's scale.
  - AV matmuls consume attn^T as the STATIONARY operand: out[q, d] +=
    a_t.T @ V-tile with free dim = 64 (bf16 V keeps 1.0 cycles/row below
    the 256-row f32r threshold), so the AV stream charges 64 rows/matmul
    instead of 512: PE attention cost drops 2x.  All 8 series of a (qc,
    hp) round accumulate into ONE PSUM bank ([128, 4, 128]); exactly one
    start=True (zeroes the 2KB zero-region) and one stop=True.
  - AV emission is readiness-ordered and flows across round boundaries
    (PSUM accumulation is order-independent), so the in-order PE stream
    never stalls on a late sigmoid; output evacuation is deferred into the
    next round and runs on DVE, with the DMA in natural [tok, feat] layout.
  - End-to-end max rel err ~5e-3 (fp8 score operands + bf16 attn/V).
  - Nonzero biases are folded in by augmenting the contraction dim with a
    ones-row (host-side, KT=9); with zero biases (the spec'd case) no
    padding is used.
"""

import sys

if "/opt/trn_rl_repo" not in sys.path:
    sys.path.insert(0, "/opt/trn_rl_repo")

from contextlib import ExitStack

import numpy as np

import concourse.tile as tile
from concourse import bacc, mybir
from concourse.bass_utils import run_bass_kernel_spmd

B, S, D, H = 4, 2048, 1024, 16
HD = D // H  # 64
OF = D // 2  # 512 features (8 heads) per core
N_CORES = 8
P = 128
TOK_T = S // P  # 16 token tiles
QC = S // 512  # 4 query chunks of 512
HP = 4  # head pairs per core
F32 = mybir.dt.float32
F32R = mybir.dt.float32r
BF16 = mybir.dt.bfloat16
FP8 = mybir.dt.float8e4  # e4m3

# number of (kt, head) S-tile jobs per (head-pair, q-chunk) per ACTIVATE.
# 2 jobs = one 2-bank PSUM wave (1024-elem ACT instructions); 3-bank waves
# amortize ACT overhead better in the cost model but mis-executed on the
# fake_nrt path, so stay at 2.
WAVE = 2
ACT_AV_LAG = 3   # AV trail (waves) for ACT-routed sigmoids
POOL_AV_LAG = 7  # AV trail for the longer DVE->Pool chain

# When True, the projection inputs (x^T and W) are shipped and multiplied in
# bfloat16: halves the serial prefix DMA (~27 MiB -> ~13.5 MiB) at the cost of
# ~10x higher (but still small) output error. Default off: fp32/float32r
# everywhere gives ~2.5e-4 max rel err.
BF16_INPUTS = False

_cache: dict = {}

# results of the most recent run (exec time etc.), for test harnesses
last_results = None

# ---- custom fused-DVE sigmoid approximation ----
# DVE waves compute the unclamped saturated-curvature cubic
#     t = (min(x*x, CAP)*B + A)*x + 0.5
# on the raw scores x (1/sqrt(HD) folded into A/B/CAP) in ONE DVE
# instruction; the idle GPSIMD/Pool engine then applies clamp01 (it may not
# touch PSUM, but t lives in SBUF).  sigma(x/8) is approximated to ~0.0033
# weighted-rms / 0.04 max err; ACT waves use the true sigmoid.
SIG_A = 0.2411235 / 8
SIG_B = -0.0119587 / 512
SIG_CAP = 64 * 7.4870063
# k-tiles whose sigmoid waves use the DVE/Pool cubic approximation (fixed
# set so the +0.5 correction's V-mass is precomputable once).  Empty: the
# measured engine balance favors the exact ACT sigmoid for every wave (the
# standard-instruction approx chain costs more DVE time than it saves on
# ACT), and the error margin vs the 2e-2 gate stays ~4x.
APPROX_KT = ()

_SIG_OP = None


def _sigmoid_dve_op():
    global _SIG_OP
    if _SIG_OP is not None:
        return _SIG_OP
    import concourse.dve_ops as dvo
    from concourse.dve_spec import (C0, C1, C2, Spec, Src0, Src1, lower,
                                    minn, sq)
    from concourse.dve_uop import DveOpSpec

    name = "SIGTAIL_CUBIC_ANT"
    for op in dvo.OPS:
        if op.name == name:
            _SIG_OP = op
            return op
    body = (minn(sq(Src0), C2) * C1 + C0) * Src0 + Src1

    def ref(in0, in1, c0, c1, c2):
        x = in0.astype(np.float32)
        return (np.minimum(x * x, c2) * c1 + c0) * x + in1

    spec = Spec(body=body, reference=ref)
    opcode = max(dvo._SUB_OPCODE_FOR_NAME.values()) + 1
    shas = {}
    for ver in ("v3", "v4"):
        try:
            uops = lower(spec, ver=ver)
        except ValueError:
            continue
        shas[ver] = DveOpSpec(name=name, opcode=opcode, uops=uops,
                              rd1_en=True).sha(ver)
    op = dvo.DveOp(name, spec, subdim=False, uops_sha=shas)
    dvo.OPS.append(op)
    dvo.CUSTOM_DVE_SPECS[name] = spec
    dvo._SUB_OPCODE_FOR_NAME[name] = opcode
    _SIG_OP = op
    return op


def _build(KT: int):
    """Build the SPMD Bass program. KT = contraction k-tiles (8, or 9 when
    biases are folded in via an augmented ones-row)."""
    nc = bacc.Bacc("TRN2", target_bir_lowering=False, debug=False,
                   num_devices=N_CORES, name="mha_sig")

    KA = KT * P  # augmented contraction size
    XDT = mybir.dt.bfloat16 if BF16_INPUTS else F32R
    xq = nc.dram_tensor("xq", [KA, S], XDT, kind="ExternalInput")
    xk = nc.dram_tensor("xk", [KA, S], XDT, kind="ExternalInput")
    xv = nc.dram_tensor("xv", [KA, S], XDT, kind="ExternalInput")
    wq = nc.dram_tensor("wq", [KA, OF], XDT, kind="ExternalInput")
    wk = nc.dram_tensor("wk", [KA, OF], XDT, kind="ExternalInput")
    wv = nc.dram_tensor("wv", [KA, OF], XDT, kind="ExternalInput")
    # natural [token, feature] layout: the AV matmuls produce out[q, d]
    o_nat = nc.dram_tensor("o_nat", [S, OF], F32, kind="ExternalOutput")
    # view for per-(qc, hp, qs) output stores: [p, qc, qs, hp, j]
    o_nat_r = o_nat.rearrange("(qc qs p) (hp j) -> p qc qs hp j",
                              qs=4, p=P, hp=HP)

    # the augmented (KT=9) layout is bigger; drop x-chunk buffering to fit
    xbufs = 3 if KT == 8 else 2

    xq_r = xq.rearrange("(kt p) t -> p kt t", p=P)
    xk_r = xk.rearrange("(kt p) t -> p kt t", p=P)
    xv_r = xv.rearrange("(kt p) t -> p kt t", p=P)

    with tile.TileContext(nc) as tc:
        with ExitStack() as ctx:
            persist = ctx.enter_context(tc.tile_pool(name="persist", bufs=1))
            wpool = ctx.enter_context(tc.tile_pool(name="wpool", bufs=1))
            xpool = ctx.enter_context(tc.tile_pool(name="xpool", bufs=2))
            ps_pool = ctx.enter_context(
                tc.tile_pool(name="ps_pool", bufs=2, space="PSUM"))
            apool = ctx.enter_context(tc.tile_pool(name="apool", bufs=6))
            opool = ctx.enter_context(tc.tile_pool(name="opool", bufs=2))

            # constants for the +0.5 correction of centered approx waves:
            # halfcol sums 0.5*V over the approx k-tiles, ones_row broadcasts
            # the result over all 128 q-partitions of a round
            halfcol = persist.tile([P, 1], BF16)
            nc.vector.memset(halfcol[:], 0.5)
            ones_row = persist.tile([1, P], BF16)
            nc.vector.memset(ones_row[:], 1.0)
            vpart_sb = persist.tile([1, OF], BF16)

            def emit_sigmoid(a_t, st, nwave, kt, tag):
                """ACT waves compute the true sigmoid; waves with kt in
                APPROX_KT get the saturated-curvature cubic via standard
                DVE/Pool instructions, producing CENTERED attn-0.5 in
                [-1/2, 1/2] (the +0.5 mass is restored by a rank-1
                correction matmul per round).  Returns True if ACT."""
                sig_ctr_started[0] = True
                if kt not in APPROX_KT:
                    nc.scalar.activation(
                        out=a_t[:, :nwave, :],
                        in_=st[:, :nwave, :],
                        func=mybir.ActivationFunctionType.Sigmoid,
                        scale=1.0 / np.sqrt(HD).item(),
                    )
                    return True
                y_cp = apool.tile([P, WAVE, 512], BF16, tag="y_cp", bufs=2,
                                  name=f"ycp_{tag}")
                nc.vector.tensor_copy(out=y_cp[:, :nwave, :],
                                      in_=st[:, :nwave, :])
                u_sq = apool.tile([P, WAVE, 512], BF16, tag="u_sq", bufs=2,
                                  name=f"usq_{tag}")
                nc.vector.tensor_tensor(
                    out=u_sq[:, :nwave, :], in0=y_cp[:, :nwave, :],
                    in1=y_cp[:, :nwave, :], op=mybir.AluOpType.mult)
                w_t = apool.tile([P, WAVE, 512], BF16, tag="w_t", bufs=2,
                                 name=f"wt_{tag}")
                nc.vector.tensor_scalar(
                    out=w_t[:, :nwave, :], in0=u_sq[:, :nwave, :],
                    scalar1=SIG_CAP, scalar2=SIG_B,
                    op0=mybir.AluOpType.min, op1=mybir.AluOpType.mult)
                v_t = apool.tile([P, WAVE, 512], BF16, tag="v_t", bufs=2,
                                 name=f"vt_{tag}")
                nc.vector.scalar_tensor_tensor(
                    out=v_t[:, :nwave, :], in0=w_t[:, :nwave, :],
                    scalar=SIG_A, in1=st[:, :nwave, :],
                    op0=mybir.AluOpType.add, op1=mybir.AluOpType.mult)
                nc.gpsimd.tensor_scalar(
                    out=a_t[:, :nwave, :], in0=v_t[:, :nwave, :],
                    scalar1=-0.5, scalar2=0.5,
                    op0=mybir.AluOpType.max, op1=mybir.AluOpType.min)
                return False

            # --- persistent weights + projection outputs ---
            # (each W is DMA'd right before the projection phase that uses it
            # so the serial prefix DMA stream isn't front-loaded with all
            # three weight tensors)
            wk_sb = persist.tile([P, KT, OF], XDT)
            wk_r = wk.rearrange("(kt p) n -> p kt n", p=P)
            for m in range(HP):
                # 4 column-chunk DMAs: the first K projection series only
                # needs columns 0:128, so the PE starts ~5us earlier
                nc.sync.dma_start(wk_sb[:, :, m * P:(m + 1) * P],
                                  wk_r[:, :, m * P:(m + 1) * P])
            wv_sb = persist.tile([P, KT, OF], XDT)
            wq_sb = persist.tile([P, KT, OF], XDT)

            # K^T / Q^T in fp8, pair-packed for DoubleRow score matmuls:
            # partition p = hsub*32 + pr (hsub = head-in-group), slot i in
            # the free dim; element [p, g, i, t] = X^T[head g*4+hsub,
            # d = 2*pr + i, t].  The projection W columns are permuted
            # host-side so each (g, i) series lands directly in this layout.
            # V: [tok-in-tile, kt, of] bf16.
            k_dr = persist.tile([P, 2, 2, S], FP8)
            q_dr = persist.tile([P, 2, 2, S], FP8)
            v_sb = persist.tile([P, TOK_T, OF], BF16)

            # projection PSUM accumulators ride in the scores-tag slots (the
            # attention score tiles are idle during the projection phase and
            # PSUM has no room for dedicated accumulators once scores run
            # triple-buffered)
            pacc_ctr = [0]

            def proj_acc(name):
                pacc_ctr[0] += 1
                t = ps_pool.tile([P, WAVE, 512], F32, tag="scores", bufs=3,
                                 name=name)
                return t

            copy_ctr = [0]
            sig_ctr_started = [False]  # set once attention waves begin

            def evac_copy(out, in_):
                # PSUM evacuation: DVE-heavy (ACT is the sigmoid bottleneck);
                # every third copy goes to ACT only during the projection
                # prefix, where ACT would otherwise idle
                copy_ctr[0] += 1
                if sig_ctr_started[0] or copy_ctr[0] % 3:
                    nc.vector.tensor_copy(out=out, in_=in_)
                else:
                    nc.scalar.copy(out=out, in_=in_)

            def proj_transposed(x_r, w_sb, dst_of, tok0, label,
                                width=512):
                """dst[:, m, tok0:tok0+width] = (W-slice).T @ x-chunk
                ([of, tok])."""
                x_tile = xpool.tile([P, KT, width], XDT, tag="xchunk",
                                    bufs=xbufs, name=f"x_{label}_{tok0}")
                nc.sync.dma_start(
                    x_tile[:],
                    x_r[:, :, tok0:tok0 + width])
                for m in range(HP):
                    ps = proj_acc(f"ps_{label}_{tok0}_{m}")
                    for kt in range(KT):
                        nc.tensor.matmul(
                            ps[:, 0, :width],
                            lhsT=w_sb[:, kt, m * P:(m + 1) * P],
                            rhs=x_tile[:, kt, :],
                            start=(kt == 0),
                            stop=(kt == KT - 1),
                        )
                    evac_copy(dst_of(m), ps[:, 0, :width])

            def proj_v(hc_idx):
                """v_sb[:, hc*2+m, :] = x-half-chunk.T @ Wv  ([tok, of])."""
                x_tile = xpool.tile([P, KT, 256], XDT, tag="xchunk", bufs=xbufs,
                                    name=f"x_v_{hc_idx}")
                nc.sync.dma_start(x_tile[:],
                                  xv_r[:, :, hc_idx * 256:(hc_idx + 1) * 256])
                for m in range(2):
                    ps = proj_acc(f"ps_v_{hc_idx}_{m}")
                    for kt in range(KT):
                        nc.tensor.matmul(
                            ps[:, 0, :],
                            lhsT=x_tile[:, kt, m * P:(m + 1) * P],
                            rhs=wv_sb[:, kt, :],
                            start=(kt == 0),
                            stop=(kt == KT - 1),
                        )
                    evac_copy(v_sb[:, hc_idx * 2 + m, :], ps[:, 0, :])

            # K projections and the first Q chunk gate attention round 0 and
            # run as a serial prefix; V and Q chunks 1-3 become injectable
            # tasks drained into the early attention rounds so the sigmoid
            # engines never sit idle behind the projection phase.
            # the first two K chunks are half-width so the very first
            # projection matmul is gated on a 3.2us DMA instead of 6.3us
            for tok0, kw in ((0, 256), (256, 256), (512, 512),
                             (1024, 512), (1536, 512)):
                proj_transposed(
                    xk_r, wk_sb,
                    lambda m, _t=tok0, _w=kw: k_dr[:, m // 2, m % 2,
                                                   _t:_t + _w],
                    tok0, "k", width=kw)
            for m in range(HP):
                nc.sync.dma_start(
                    wq_sb[:, :, m * P:(m + 1) * P],
                    wq.rearrange("(kt p) n -> p kt n", p=P)[:, :,
                                                           m * P:(m + 1) * P])
            proj_transposed(
                xq_r, wq_sb,
                lambda m: q_dr[:, m // 2, m % 2, 0:512], 0, "q")
            nc.sync.dma_start(wv_sb[:], wv.rearrange("(kt p) n -> p kt n", p=P))

            # --- deferred projection task queue ---
            proj_tasks = []

            def v_dma_task(hc_idx):
                def run():
                    x_tile = xpool.tile([P, KT, 256], XDT, tag="xchunk",
                                        bufs=xbufs, name=f"x_v_{hc_idx}")
                    nc.sync.dma_start(
                        x_tile[:],
                        xv_r[:, :, hc_idx * 256:(hc_idx + 1) * 256])
                    v_tiles[hc_idx] = x_tile
                return run

            def v_series_task(hc_idx, m):
                def run():
                    x_tile = v_tiles[hc_idx]
                    ps = proj_acc(f"ps_v_{hc_idx}_{m}")
                    for kt in range(KT):
                        nc.tensor.matmul(
                            ps[:, 0, :],
                            lhsT=x_tile[:, kt, m * P:(m + 1) * P],
                            rhs=wv_sb[:, kt, :],
                            start=(kt == 0),
                            stop=(kt == KT - 1),
                        )
                    evac_copy(v_sb[:, hc_idx * 2 + m, :], ps[:, 0, :])
                return run

            def q_dma_task(tc_idx):
                def run():
                    x_tile = xpool.tile([P, KT, 512], XDT, tag="xchunk",
                                        bufs=xbufs, name=f"x_q_{tc_idx}")
                    nc.sync.dma_start(
                        x_tile[:],
                        xq_r[:, :, tc_idx * 512:(tc_idx + 1) * 512])
                    q_tiles[tc_idx] = x_tile
                return run

            q_accs: dict = {}

            def q_series_task(tc_idx, m, half):
                # half-series granules (~0.85us of PE each) so a drained
                # projection never starves the ACT sigmoid stream through
                # the 3-slot score buffer
                def run():
                    x_tile = q_tiles[tc_idx]
                    if half == 0:
                        ps = proj_acc(f"ps_q_{tc_idx}_{m}")
                        q_accs[(tc_idx, m)] = ps
                    else:
                        ps = q_accs[(tc_idx, m)]
                    lo = 0 if half == 0 else KT // 2
                    hi = KT // 2 if half == 0 else KT
                    for kt in range(lo, hi):
                        nc.tensor.matmul(
                            ps[:, 0, :],
                            lhsT=wq_sb[:, kt, m * P:(m + 1) * P],
                            rhs=x_tile[:, kt, :],
                            start=(kt == 0),
                            stop=(kt == KT - 1),
                        )
                    if half == 1:
                        evac_copy(
                            q_dr[:, m // 2, m % 2,
                                 tc_idx * 512:(tc_idx + 1) * 512],
                            ps[:, 0, :])
                return run

            def vpart_task():
                # vpart[of] = 0.5 * sum over approx k-tiles of V rows; the
                # per-round correction matmuls broadcast it across q
                if not APPROX_KT:
                    return
                ps = proj_acc("ps_vpart")
                for i, kt in enumerate(APPROX_KT):
                    nc.tensor.matmul(
                        ps[0:1, 0, :],
                        lhsT=halfcol[:, :],
                        rhs=v_sb[:, kt, :],
                        start=(i == 0),
                        stop=(i == len(APPROX_KT) - 1),
                    )
                nc.vector.tensor_copy(out=vpart_sb[:, :], in_=ps[0:1, 0, :])

            v_tiles: dict = {}
            q_tiles: dict = {}
            for hc in range(8):
                proj_tasks.append(v_dma_task(hc))
                proj_tasks.append(v_series_task(hc, 0))
                proj_tasks.append(v_series_task(hc, 1))
            proj_tasks.append(vpart_task)
            for tc_idx in range(1, QC):
                proj_tasks.append(q_dma_task(tc_idx))
                for m in range(HP):
                    proj_tasks.append(q_series_task(tc_idx, m, 0))
                    proj_tasks.append(q_series_task(tc_idx, m, 1))

            def drain_proj(n):
                for _ in range(n):
                    if proj_tasks:
                        proj_tasks.pop(0)()

            # jobs per (hp, qc): (kt, head) pairs
            jobs = [(kt, h) for kt in range(TOK_T) for h in range(2)]
            waves = [jobs[i:i + WAVE] for i in range(0, len(jobs), WAVE)]

            # deferred output evacuation: (o_acc, qc, hp) whose copy + DMA
            # are emitted a few waves into the NEXT round, so the engine
            # queues never head-of-line-block the next round's sigmoids
            pending_out = []

            def flush_pending_out():
                while pending_out:
                    po_acc, po_qc, po_hp = pending_out.pop(0)
                    o_sb = opool.tile([P, 4, P], F32, tag="o_sb",
                                      name=f"osb_{po_qc}_{po_hp}")
                    nc.vector.tensor_copy(out=o_sb[:], in_=po_acc[:])
                    nc.sync.dma_start(o_nat_r[:, po_qc, :, po_hp, :], o_sb[:])

            # --- attention rounds with cross-round AV flow ---
            # AV matmuls drain by expected a_t readiness (global wave index),
            # flowing freely across round boundaries so a late approx-chain
            # a_t never stalls the round tail: the next round's scores and
            # sigmoids proceed while the stragglers land.
            pending = []  # (ready_gw, seq, wave, a_t, rctx)
            seq_ctr = [0]

            def emit_avs(wave, a_t, rctx):
                o_acc, r_hp = rctx["o_acc"], rctx["hp"]
                for j, (kt, h) in enumerate(wave):
                    for qs in range(4):
                        # out[q, d] += attn^T-tile.T @ V-tile
                        nc.tensor.matmul(
                            o_acc[:, qs, h * HD:(h + 1) * HD],
                            lhsT=a_t[:, j, qs * P:(qs + 1) * P],
                            rhs=v_sb[:, kt,
                                     r_hp * P + h * HD:
                                     r_hp * P + (h + 1) * HD],
                            start=(rctx["emitted"] == 0),
                            stop=(not APPROX_KT
                                  and rctx["emitted"] == rctx["total"] - 1),
                        )
                        rctx["emitted"] += 1
                if rctx["emitted"] == rctx["total"]:
                    if APPROX_KT:
                        # restore the +0.5 mass the centered approx waves
                        # dropped (rank-1 over q); last one closes the group
                        for qs in range(4):
                            nc.tensor.matmul(
                                o_acc[:, qs, :],
                                lhsT=ones_row[0:1, :],
                                rhs=vpart_sb[0:1, r_hp * P:(r_hp + 1) * P],
                                start=False,
                                stop=(qs == 3),
                            )
                    pending_out.append((o_acc, rctx["qc"], r_hp))

            def service_pending(now_gw):
                pending.sort()
                while pending and pending[0][0] <= now_gw:
                    _, _, w_, at_, rctx_ = pending.pop(0)
                    emit_avs(w_, at_, rctx_)

            gw = 0
            for qc in range(QC):
                for hp in range(HP):
                    o_acc = ps_pool.tile([P, 4, P], F32, tag="oacc", bufs=2,
                                         name=f"oacc_{qc}_{hp}")
                    rctx = {"o_acc": o_acc, "qc": qc, "hp": hp,
                            "emitted": 0, "total": len(waves) * WAVE * 4}
                    for wi, wave in enumerate(waves):
                        st = ps_pool.tile([P, WAVE, 512], F32, tag="scores",
                                       bufs=3, name=f"st_{qc}_{hp}_{wi}")
                        for j, (kt, h) in enumerate(wave):
                            # scores^T tile: [k-tokens, q-tokens] for head
                            # 2hp+h via fp8 DoubleRow: contraction d = 64
                            # packed as 32 partitions x 2 slots
                            g, hsub = hp // 2, 2 * (hp % 2) + h
                            nc.tensor.matmul(
                                st[:, j, :],
                                lhsT=k_dr[hsub * 32:(hsub + 1) * 32, g, :,
                                          kt * P:(kt + 1) * P],
                                rhs=q_dr[hsub * 32:(hsub + 1) * 32, g, :,
                                         qc * 512:(qc + 1) * 512],
                                start=True,
                                stop=True,
                                perf_mode=mybir.MatmulPerfMode.DoubleRow,
                                tile_position=(hsub * 32, 0),
                            )
                        a_t = apool.tile([P, WAVE, 512], BF16, tag="a_t",
                                         bufs=14, name=f"a_{qc}_{hp}_{wi}")
                        is_act = emit_sigmoid(a_t, st, len(wave), wave[0][0],
                                              f"{qc}_{hp}_{wi}")
                        # drain deferred projections: V inside round 0 (its
                        # AVs need kt progressively), Q chunks trickled
                        # through later rounds ahead of their q-chunk
                        if qc == 0 and hp == 0:
                            if wi < 12:
                                drain_proj(2)
                        elif wi in (2, 6, 10, 14):
                            drain_proj(1)
                        if wi == 2:
                            # previous rounds' outputs leave PSUM only now:
                            # their copies never queue ahead of this round's
                            # sigmoid work on DVE
                            flush_pending_out()
                        ready = gw + (ACT_AV_LAG if is_act else POOL_AV_LAG)
                        seq_ctr[0] += 1
                        pending.append((ready, seq_ctr[0], wave, a_t, rctx))
                        service_pending(gw)
                        gw += 1
            service_pending(10 ** 9)
            flush_pending_out()

    nc.compile()
    return nc


def _prep_core_inputs(q, k, v, Wq, bq, Wk, bk, Wv, bv, KT):
    """Host-side shard + transpose. Returns in_maps for 8 cores."""
    KA = KT * P
    aug = KA > D
    if BF16_INPUTS:
        import ml_dtypes
        xdt = ml_dtypes.bfloat16
    else:
        xdt = np.float32

    def x_t(x_b):  # [S, D] -> [KA, S]
        xt = np.ascontiguousarray(x_b.T)  # [D, S]
        if not aug:
            return xt.astype(xdt)
        out = np.zeros((KA, S), xdt)
        out[:D] = xt
        out[D] = 1.0
        return out

    # Q/K weight columns are permuted so projection series m = (g, i) lands
    # directly in the pair-packed fp8 layout the DoubleRow score matmuls
    # read: series m, PSUM partition p <- of-column (g*4 + p//32)*64 +
    # 2*(p%32) + i of the core's half.
    perm = np.empty(OF, np.int64)
    for m in range(4):
        g, i = divmod(m, 2)
        p = np.arange(P)
        perm[m * P + p] = (g * 4 + p // 32) * 64 + 2 * (p % 32) + i

    def w_slice(W, b, half, permute=False):  # -> [KA, OF]
        ws = W[:, half * OF:(half + 1) * OF]
        bs = b[half * OF:(half + 1) * OF]
        if permute:
            ws = ws[:, perm]
            bs = bs[perm]
        if not aug:
            return np.ascontiguousarray(ws).astype(xdt)
        out = np.zeros((KA, OF), xdt)
        out[:D] = ws
        out[D] = bs
        return out

    xts = {}
    in_maps = []
    for c in range(N_CORES):
        b, half = divmod(c, 2)
        if b not in xts:
            xts[b] = (x_t(q[b]), x_t(k[b]), x_t(v[b]))
        xq_c, xk_c, xv_c = xts[b]
        in_maps.append({
            "xq": xq_c,
            "xk": xk_c,
            "xv": xv_c,
            "wq": w_slice(Wq, bq, half, permute=True),
            "wk": w_slice(Wk, bk, half, permute=True),
            "wv": w_slice(Wv, bv, half),
        })
    return in_maps


def kernel(q, k, v, Wq, bq, Wk, bk, Wv, bv):
    global last_results
    q = np.ascontiguousarray(np.asarray(q, np.float32))
    k = np.ascontiguousarray(np.asarray(k, np.float32))
    v = np.ascontiguousarray(np.asarray(v, np.float32))
    Wq = np.asarray(Wq, np.float32)
    Wk = np.asarray(Wk, np.float32)
    Wv = np.asarray(Wv, np.float32)
    bq = np.asarray(bq, np.float32)
    bk = np.asarray(bk, np.float32)
    bv = np.asarray(bv, np.float32)

    aug = any(np.any(b_) for b_ in (bq, bk, bv))
    KT = (D // P) + (1 if aug else 0)

    key = (KT, BF16_INPUTS)
    if key not in _cache:
        _cache[key] = _build(KT)
    nc = _cache[key]

    in_maps = _prep_core_inputs(q, k, v, Wq, bq, Wk, bk, Wv, bv, KT)
    res = run_bass_kernel_spmd(nc, in_maps, core_ids=list(range(N_CORES)))
    last_results = res

    out = np.empty((B, S, D), np.float32)
    for c in range(N_CORES):
        b, half = divmod(c, 2)
        out[b, :, half * OF:(half + 1) * OF] = res.results[c]["o_nat"]
    return out

